# revision 1
# baseline (speedup 1.0000x reference)
"""DLinear Trainium2 kernel (nn_DLinear_45990509805636).

Math: with T=17 and KERNEL_SIZE=37 (PAD=18), every moving-average window
covers the whole sequence plus replicated edges, so

    trend[b,t,:] = (S + (18-t)*x0 + (t+2)*x16) / 37,   S = sum_t x[:,t,:]
    out = seasonal @ Ws[t] + trend @ Wt[t] + (bs+bt)[t]
        = x_t @ Ws[t] + trend_raw_t @ Wd[t] + bias[t],
    Wd = (Wt - Ws)/37 (host-folded), trend_raw_t = P + t*Q,
    P = S + 18*x0 + 2*x16, Q = x16 - x0.

Device per core (batch shard of 512 rows):
  - x.T resident in SBUF as [c%128, t, c//128, b]
  - S/P/Q/trend on DVE (bf16, 2x/4x modes)
  - per (b-tile, t): PSUM group = K=1 bias-broadcast matmul + 4 x@Ws
    matmuls (+ 4 trend@Wd matmuls, either same group or a second group
    joined by a DVE add when PHASE_SPLIT) -> ACT/DVE copy -> DMA out.

Sharding: data-parallel over batch, 8 cores x 512 rows; weights replicated.
"""

import os
import sys

sys.path.insert(0, "/opt/trn_rl_repo")

import numpy as np
import ml_dtypes

from concourse import bacc
import concourse.mybir as mybir
import concourse.tile as tile
from concourse.bass_utils import run_bass_kernel_spmd

dt = mybir.dt

B, T, C, D = 4096, 17, 512, 512
NCORES = 8
BC = B // NCORES          # 512 batch rows per core
KC = C // 128             # 4 contraction chunks
JB = BC // 128            # 4 output-row tiles per core

MODE = os.environ.get("DLINEAR_MODE", "bf16")


def build_bf16():
    # DRAM layouts are host-prepared so every DMA descriptor is a >=4KB
    # contiguous run: xt/wst/wdt are [t, c%128, c//128, {b,d}].
    idt = dt.bfloat16
    nc = bacc.Bacc(None, target_bir_lowering=False, name="dlinear_bf16")
    xt = nc.dram_tensor("xt", [T, 128, KC, BC], idt, kind="ExternalInput")
    wst = nc.dram_tensor("wst", [T, 128, KC, D], idt, kind="ExternalInput")
    wdt = nc.dram_tensor("wdt", [T, 128, KC, D], idt, kind="ExternalInput")
    bias = nc.dram_tensor("bias", [1, T * D], idt, kind="ExternalInput")
    out = nc.dram_tensor("out", [BC, T, D], dt.float16, kind="ExternalOutput")

    with tile.TileContext(nc) as tc:
        with (
            tc.tile_pool(name="xres", bufs=1) as xres,
            tc.tile_pool(name="consts", bufs=1) as consts,
            tc.tile_pool(name="stats", bufs=1) as stats,
            tc.tile_pool(name="wsbuf", bufs=4) as wsbuf,
            tc.tile_pool(name="wdbuf", bufs=3) as wdbuf,
            tc.tile_pool(name="tbuf", bufs=2) as tbuf,
            tc.tile_pool(name="abuf", bufs=44) as abuf,
            tc.tile_pool(name="obuf", bufs=4) as obuf,
            tc.tile_pool(name="psum_a", bufs=4, space="PSUM") as psum_a,
            tc.tile_pool(name="psum_b", bufs=4, space="PSUM") as psum_b,
        ):
            # SP dispatches dma_starts in-order at ~0.7us each and each
            # dma_start rides ONE ~20GB/s queue, so: few dispatches, sized
            # 128-512KB, issued in consumption order; out-stores dispatch
            # from the (otherwise idle) ACT sequencer.
            ones = consts.tile([1, 128], idt)
            nc.vector.memset(ones, 1.0)
            bsb = consts.tile([1, T * D], idt)
            nc.sync.dma_start(bsb, bias[:])

            xsb = xres.tile([128, T, KC, BC], idt)

            def emit_phase_a(t, wss, j):
                # bias + x@Ws; independent of S/trend, fills the prologue
                psa = psum_a.tile([128, D], dt.float32, tag="psa", name="psa")
                nc.tensor.matmul(psa, ones, bsb[:, t * D:(t + 1) * D],
                                 start=True, stop=False)
                for k in range(KC):
                    nc.tensor.matmul(
                        psa, xsb[:, t, k, j * 128:(j + 1) * 128], wss[:, k],
                        start=False, stop=(k == KC - 1),
                    )
                outa = abuf.tile([128, D], idt, tag="outa", name="outa")
                nc.scalar.copy(outa, psa)
                return outa

            PRE_T = 12
            ws_pre = {}
            for t in range(4):
                wss = wsbuf.tile([128, KC, D], idt, tag="ws", name="ws")
                nc.sync.dma_start(wss[:, 0:2], wst[t, :, 0:2])
                nc.sync.dma_start(wss[:, 2:4], wst[t, :, 2:4])
                ws_pre[t] = wss
            for t in range(T):
                nc.sync.dma_start(xsb[:, t, 0:2], xt[t, :, 0:2])
                nc.sync.dma_start(xsb[:, t, 2:4], xt[t, :, 2:4])
            # early wd so phase-B(0..3) isn't gated on late SP dispatch
            wd_pre = {}
            for t in range(3):
                wds = wdbuf.tile([128, KC, D], idt, tag="wd", name="wd")
                nc.sync.dma_start(wds[:, 0:2], wdt[t, :, 0:2])
                nc.sync.dma_start(wds[:, 2:4], wdt[t, :, 2:4])
                wd_pre[t] = wds
            for t in range(4, PRE_T):
                wss = wsbuf.tile([128, KC, D], idt, tag="ws", name="ws")
                nc.sync.dma_start(wss[:, 0:2], wst[t, :, 0:2])
                nc.sync.dma_start(wss[:, 2:4], wst[t, :, 2:4])
                ws_pre[t] = wss

            outa_pre = {}
            for t in range(PRE_T):
                for j in range(JB):
                    outa_pre[(t, j)] = emit_phase_a(t, ws_pre[t], j)

            # S/P/Q in bf16 on full [128, KC*BC] views (DVE 2x TT, 4x TS)
            S = stats.tile([128, KC, BC], idt)
            P = stats.tile([128, KC, BC], idt)
            Q = stats.tile([128, KC, BC], idt)
            nc.vector.tensor_tensor(S[:], xsb[:, 0], xsb[:, 1], mybir.AluOpType.add)
            for t in range(2, T):
                nc.vector.tensor_tensor(S[:], S[:], xsb[:, t], mybir.AluOpType.add)
            nc.vector.scalar_tensor_tensor(P[:], xsb[:, 0], 18.0, S[:],
                                           mybir.AluOpType.mult, mybir.AluOpType.add)
            nc.vector.scalar_tensor_tensor(P[:], xsb[:, 16], 2.0, P[:],
                                           mybir.AluOpType.mult, mybir.AluOpType.add)
            nc.vector.tensor_tensor(Q[:], xsb[:, 16], xsb[:, 0], mybir.AluOpType.subtract)

            osb_cur = {}
            for t in range(T):
                if t >= PRE_T:
                    wss = wsbuf.tile([128, KC, D], idt, tag="ws", name="ws")
                    nc.sync.dma_start(wss[:, 0:2], wst[t, :, 0:2])
                    nc.sync.dma_start(wss[:, 2:4], wst[t, :, 2:4])
                    ws_pre[t] = wss
                if t < 3:
                    wds = wd_pre.pop(t)
                else:
                    wds = wdbuf.tile([128, KC, D], idt, tag="wd", name="wd")
                    nc.sync.dma_start(wds[:, 0:2], wdt[t, :, 0:2])
                    nc.sync.dma_start(wds[:, 2:4], wdt[t, :, 2:4])

                trend = tbuf.tile([128, KC, BC], idt, tag="trend", name="trend")
                if t == 0:
                    nc.vector.tensor_copy(trend[:], P[:])
                else:
                    nc.vector.tensor_scalar_mul(trend[:], Q[:], float(t))
                    nc.vector.tensor_tensor(trend[:], trend[:], P[:], mybir.AluOpType.add)

                for j in range(JB):
                    if t < PRE_T:
                        outa = outa_pre.pop((t, j))
                    else:
                        outa = emit_phase_a(t, ws_pre[t], j)
                    psb = psum_b.tile([128, D], dt.float32, tag="psb", name="psb")
                    for k in range(KC):
                        nc.tensor.matmul(
                            psb, trend[:, k, j * 128:(j + 1) * 128], wds[:, k],
                            start=(k == 0), stop=(k == KC - 1),
                        )
                    # pair the stores: out[b, t-1:t+1, :] is contiguous, so
                    # buffer two tokens per [128, 2, D] tile and store once
                    # (dispatched from ACT to keep SP free for loads)
                    if t == T - 1:
                        osb = obuf.tile([128, 1, D], dt.float16, tag="osb1", name="osb1")
                        nc.vector.scalar_tensor_tensor(
                            osb[:, 0], psb, 1.0, outa,
                            mybir.AluOpType.mult, mybir.AluOpType.add,
                        )
                        nc.scalar.dma_start(
                            out[j * 128:(j + 1) * 128, t:t + 1, :], osb)
                    else:
                        if t % 2 == 0:
                            osb = obuf.tile([128, 2, D], dt.float16, tag="osb", name="osb")
                            osb_cur[j] = osb
                        else:
                            osb = osb_cur[j]
                        nc.vector.scalar_tensor_tensor(
                            osb[:, t % 2], psb, 1.0, outa,
                            mybir.AluOpType.mult, mybir.AluOpType.add,
                        )
                        if t % 2 == 1:
                            nc.scalar.dma_start(
                                out[j * 128:(j + 1) * 128, t - 1:t + 1, :], osb)
    nc.compile()
    return nc


def build_f32r():
    """x streamed twice in f32r; fp32-grade accuracy (~1.5e-4)."""
    idt = dt.float32r
    nc = bacc.Bacc(None, target_bir_lowering=False, name="dlinear_f32r")
    xt = nc.dram_tensor("xt", [T, C, BC], idt, kind="ExternalInput")
    wst = nc.dram_tensor("wst", [T, C, D], idt, kind="ExternalInput")
    wdt = nc.dram_tensor("wdt", [T, C, D], idt, kind="ExternalInput")
    bias = nc.dram_tensor("bias", [1, T * D], dt.bfloat16, kind="ExternalInput")
    out = nc.dram_tensor("out", [BC, T, D], dt.float32, kind="ExternalOutput")

    with tile.TileContext(nc) as tc:
        with (
            tc.tile_pool(name="consts", bufs=1) as consts,
            tc.tile_pool(name="stats", bufs=1) as stats,
            tc.tile_pool(name="spass", bufs=2) as spass,
            tc.tile_pool(name="xbuf", bufs=3) as xbuf,
            tc.tile_pool(name="wsbuf", bufs=3) as wsbuf,
            tc.tile_pool(name="wdbuf", bufs=3) as wdbuf,
            tc.tile_pool(name="tbuf", bufs=2) as tbuf,
            tc.tile_pool(name="obuf", bufs=8) as obuf,
            tc.tile_pool(name="psum", bufs=8, space="PSUM") as psum,
        ):
            ones = consts.tile([1, 128], dt.bfloat16)
            nc.vector.memset(ones, 1.0)
            bsb = consts.tile([1, T * D], dt.bfloat16)
            nc.sync.dma_start(bsb, bias[:])

            S = stats.tile([128, KC, BC], dt.float32)
            P = stats.tile([128, KC, BC], dt.float32)
            Q = stats.tile([128, KC, BC], dt.float32)
            for k in range(KC):
                xk = spass.tile([128, T, BC], idt, tag="xk")
                nc.sync.dma_start(
                    xk, xt[:, k * 128:(k + 1) * 128, :].rearrange("t p b -> p t b")
                )
                nc.vector.tensor_tensor(S[:, k], xk[:, 0], xk[:, 1], mybir.AluOpType.add)
                for t in range(2, T):
                    nc.vector.tensor_tensor(S[:, k], S[:, k], xk[:, t], mybir.AluOpType.add)
                nc.vector.scalar_tensor_tensor(
                    P[:, k], xk[:, 0], 18.0, S[:, k],
                    mybir.AluOpType.mult, mybir.AluOpType.add,
                )
                nc.vector.scalar_tensor_tensor(
                    P[:, k], xk[:, 16], 2.0, P[:, k],
                    mybir.AluOpType.mult, mybir.AluOpType.add,
                )
                nc.vector.scalar_tensor_tensor(
                    Q[:, k], xk[:, 0], -1.0, xk[:, 16],
                    mybir.AluOpType.mult, mybir.AluOpType.add,
                )

            for t in range(T):
                xts = xbuf.tile([128, KC, BC], idt, tag="xts")
                nc.sync.dma_start(xts, xt[t].rearrange("(k p) b -> p k b", p=128))
                wss = wsbuf.tile([128, KC, D], idt, tag="ws")
                nc.sync.dma_start(wss, wst[t].rearrange("(k p) d -> p k d", p=128))
                wds = wdbuf.tile([128, KC, D], idt, tag="wd")
                nc.sync.dma_start(wds, wdt[t].rearrange("(k p) d -> p k d", p=128))
                trend = tbuf.tile([128, KC, BC], idt, tag="trend")
                nc.vector.scalar_tensor_tensor(
                    trend[:], Q[:], float(t), P[:],
                    mybir.AluOpType.mult, mybir.AluOpType.add,
                )
                for j in range(JB):
                    ps = psum.tile([128, D], dt.float32, tag="ps")
                    nc.tensor.matmul(ps, ones, bsb[:, t * D:(t + 1) * D],
                                     start=True, stop=False)
                    for k in range(KC):
                        nc.tensor.matmul(
                            ps, xts[:, k, j * 128:(j + 1) * 128], wss[:, k],
                            start=False, stop=False,
                        )
                    for k in range(KC):
                        nc.tensor.matmul(
                            ps, trend[:, k, j * 128:(j + 1) * 128], wds[:, k],
                            start=False, stop=(k == KC - 1),
                        )
                    osb = obuf.tile([128, D], dt.float32, tag="osb")
                    nc.scalar.copy(osb, ps)
                    nc.sync.dma_start(out[j * 128:(j + 1) * 128, t, :], osb)
    nc.compile()
    return nc


_NC_CACHE = {}


def _get_nc(mode):
    if mode not in _NC_CACHE:
        _NC_CACHE[mode] = build_bf16() if mode == "bf16" else build_f32r()
    return _NC_CACHE[mode]


def kernel(x, W_seasonal, b_seasonal, W_trend, b_trend, _trace=False):
    mode = MODE
    npdt = ml_dtypes.bfloat16 if mode == "bf16" else np.float32
    nc = _get_nc(mode)

    def to_tpkd(w):  # [T, D, C] -> [T, 128, KC, D] (c-major on partitions)
        wt = w.transpose(0, 2, 1).reshape(T, KC, 128, D)
        return np.ascontiguousarray(wt.transpose(0, 2, 1, 3))

    if mode == "bf16":
        wst = to_tpkd(W_seasonal).astype(npdt)
        wdt = to_tpkd((W_trend - W_seasonal) / 37.0).astype(npdt)
    else:
        wst = np.ascontiguousarray(W_seasonal.transpose(0, 2, 1)).astype(npdt)
        wdt = np.ascontiguousarray(
            ((W_trend - W_seasonal) / 37.0).transpose(0, 2, 1)
        ).astype(npdt)
    bias = (b_seasonal + b_trend).reshape(1, T * D).astype(ml_dtypes.bfloat16)

    in_maps = []
    for i in range(NCORES):
        xs = x[i * BC:(i + 1) * BC]                    # [BC, T, C]
        if mode == "bf16":
            # [T, C, BC] -> [T, 128, KC, BC]
            xti = xs.transpose(1, 2, 0).reshape(T, KC, 128, BC)
            xti = np.ascontiguousarray(xti.transpose(0, 2, 1, 3)).astype(npdt)
        else:
            xti = np.ascontiguousarray(xs.transpose(1, 2, 0)).astype(npdt)
        in_maps.append({"xt": xti, "wst": wst, "wdt": wdt, "bias": bias})

    res = run_bass_kernel_spmd(
        nc, in_maps, core_ids=list(range(NCORES)), trace=_trace
    )
    outp = np.concatenate([r["out"] for r in res.results], axis=0)
    if outp.dtype != np.float32:
        outp = outp.astype(np.float32)
    if _trace:
        return outp, res
    return outp


if __name__ == "__main__":
    rng = np.random.default_rng(0)
    x = rng.standard_normal((B, T, C), dtype=np.float32)
    Ws = rng.uniform(-0.04, 0.04, (T, D, C)).astype(np.float32)
    Wt = rng.uniform(-0.04, 0.04, (T, D, C)).astype(np.float32)
    bs = rng.uniform(-0.04, 0.04, (T, D)).astype(np.float32)
    bt = rng.uniform(-0.04, 0.04, (T, D)).astype(np.float32)
    o = kernel(x, Ws, bs, Wt, bt)
    print("out shape:", o.shape, o.dtype)



# revision 7
# speedup vs baseline: 1.3424x; 1.3424x over previous
"""DLinear Trainium2 kernel (nn_DLinear_45990509805636).

Math: with T=17 and KERNEL_SIZE=37 (PAD=18), every moving-average window
covers the whole sequence plus replicated edges, so

    trend[b,t,:] = (S + (18-t)*x0 + (t+2)*x16) / 37,   S = sum_t x[:,t,:]
    out = seasonal @ Ws[t] + trend @ Wt[t] + (bs+bt)[t]
        = x_t @ Ws[t] + trend_raw_t @ Wd[t] + bias[t],
    Wd = (Wt - Ws)/37 (host-folded), trend_raw_t = P + t*Q,
    P = S + 18*x0 + 2*x16, Q = x16 - x0.

Device per core (batch shard of 512 rows), all bf16 matmuls:
  - x.T resident in SBUF as [c%128, t, c//128, b]
  - S/P/Q/trend on DVE (bf16 2x modes), trend = Q*t + P in one STT
  - prologue (t < PRE): phase-split. x@Ws groups (4 MMs) fill the PE while
    x/weights stream in; ACT copies psum->outa; after trend is ready the
    trend@Wd group (4 MMs) is joined with outa by one DVE STT.
  - steady state (t >= PRE): ONE psum group of 8 MMs (4 x@Ws + 4 trend@Wd),
    ACT copies psum->f16, paired 2-token stores dispatched from ACT.
  - bias is NOT added on device: the host epilogue adds (bs+bt) during the
    f16->f32 upcast (saves 68 K=1 broadcast matmuls ~27us of PE time).

Sharding: data-parallel over batch, 8 cores x 512 rows; weights replicated.
"""

import sys

sys.path.insert(0, "/opt/trn_rl_repo")

import numpy as np
import ml_dtypes

from concourse import bacc
import concourse.mybir as mybir
import concourse.tile as tile
from concourse.bass_utils import run_bass_kernel_spmd

dt = mybir.dt

B, T, C, D = 4096, 17, 512, 512
NCORES = 8
BC = B // NCORES          # 512 batch rows per core
KC = C // 128             # 4 contraction chunks
JB = BC // 128            # 4 output-row tiles per core

PRE = 8                   # tokens handled phase-split to fill the DMA prologue


def build():
    idt = dt.bfloat16
    nc = bacc.Bacc(None, target_bir_lowering=False, name="dlinear_v2")
    # DRAM layouts host-prepared so every DMA descriptor is a >=4KB run:
    # xt/wst/wdt are [t, c%128, c//128, {b,d}].
    xt = nc.dram_tensor("xt", [T, 128, KC, BC], idt, kind="ExternalInput")
    wst = nc.dram_tensor("wst", [T, 128, KC, D], idt, kind="ExternalInput")
    wdt = nc.dram_tensor("wdt", [T, 128, KC, D], idt, kind="ExternalInput")
    out = nc.dram_tensor("out", [BC, T, D], dt.float16, kind="ExternalOutput")

    with tile.TileContext(nc) as tc:
        with (
            tc.tile_pool(name="xres", bufs=1) as xres,
            tc.tile_pool(name="stats", bufs=1) as stats,
            tc.tile_pool(name="wsbuf", bufs=PRE + 2) as wsbuf,
            tc.tile_pool(name="wdbuf", bufs=3) as wdbuf,
            tc.tile_pool(name="tbuf", bufs=3) as tbuf,
            tc.tile_pool(name="abuf", bufs=4 * PRE + 2) as abuf,
            tc.tile_pool(name="obuf", bufs=10) as obuf,
            tc.tile_pool(name="psum", bufs=8, space="PSUM") as psum,
        ):
            xsb = xres.tile([128, T, KC, BC], idt)

            # ---- DMA schedule (SP): interleave ws[t] with 2-token x loads so
            # phase-A(t) unblocks in order; wd after x so x (gating trend)
            # finishes as early as possible.
            ws_tiles = {}

            def load_ws(t):
                w = wsbuf.tile([128, KC, D], idt, tag="ws", name="ws")
                nc.sync.dma_start(w, wst[t])
                ws_tiles[t] = w

            def load_x2(t0):
                t1 = min(t0 + 2, T)
                nc.sync.dma_start(
                    xsb[:, t0:t1],
                    xt[t0:t1].rearrange("t p k b -> p t k b"),
                )

            load_ws(0)
            load_x2(0)
            load_ws(1)
            for i in range(1, 9):
                load_x2(2 * i)
                if i + 1 < PRE:
                    load_ws(i + 1)

            wd_tiles = {}

            def load_wd(t):
                w = wdbuf.tile([128, KC, D], idt, tag="wd", name="wd")
                nc.sync.dma_start(w, wdt[t])
                wd_tiles[t] = w

            for t in range(2):
                load_wd(t)

            # ---- phase-A prologue: pure x@Ws groups, ACT drains psum->outa
            def emit_phase_a(t, j):
                psa = psum.tile([128, D], dt.float32, tag="ps", name="psa")
                for k in range(KC):
                    nc.tensor.matmul(
                        psa, xsb[:, t, k, j * 128:(j + 1) * 128],
                        ws_tiles[t][:, k],
                        start=(k == 0), stop=(k == KC - 1),
                    )
                outa = abuf.tile([128, D], idt, tag="outa", name="outa")
                nc.scalar.copy(outa, psa)
                return outa

            outa_pre = {}
            for t in range(PRE):
                for j in range(JB):
                    outa_pre[(t, j)] = emit_phase_a(t, j)

            # ---- S/P/Q in bf16 on full [128, KC*BC] views (DVE 2x TT)
            S = stats.tile([128, KC, BC], idt)
            P = stats.tile([128, KC, BC], idt)
            Q = stats.tile([128, KC, BC], idt)
            nc.vector.tensor_tensor(Q[:], xsb[:, 16], xsb[:, 0], mybir.AluOpType.subtract)
            nc.vector.tensor_tensor(S[:], xsb[:, 0], xsb[:, 1], mybir.AluOpType.add)
            for t in range(2, T):
                nc.vector.tensor_tensor(S[:], S[:], xsb[:, t], mybir.AluOpType.add)
            nc.vector.scalar_tensor_tensor(P[:], xsb[:, 0], 18.0, S[:],
                                           mybir.AluOpType.mult, mybir.AluOpType.add)
            nc.vector.scalar_tensor_tensor(P[:], xsb[:, 16], 2.0, P[:],
                                           mybir.AluOpType.mult, mybir.AluOpType.add)

            def make_trend(t):
                trend = tbuf.tile([128, KC, BC], idt, tag="trend", name="trend")
                if t == 0:
                    nc.vector.tensor_copy(trend[:], P[:])
                else:
                    nc.vector.scalar_tensor_tensor(
                        trend[:], Q[:], float(t), P[:],
                        mybir.AluOpType.mult, mybir.AluOpType.add,
                    )
                return trend

            osb_cur = {}

            def store_out(t, j, osb):
                # pair the stores: out[b, t-1:t+1, :] is contiguous; dispatch
                # from ACT to keep SP free for loads
                if t % 2 == 1:
                    nc.scalar.dma_start(
                        out[j * 128:(j + 1) * 128, t - 1:t + 1, :], osb)
                elif t == T - 1:
                    nc.scalar.dma_start(
                        out[j * 128:(j + 1) * 128, t:t + 1, :], osb[:, 0:1])

            def get_osb(t, j):
                if t % 2 == 0 and t != T - 1:
                    osb = obuf.tile([128, 2, D], dt.float16, tag="osb", name="osb")
                    osb_cur[j] = osb
                    return osb, osb[:, 0]
                elif t == T - 1:
                    osb = obuf.tile([128, 2, D], dt.float16, tag="osb", name="osb")
                    return osb, osb[:, 0]
                else:
                    osb = osb_cur[j]
                    return osb, osb[:, 1]

            # ---- phase-B for prologue tokens: 4-MM trend groups + DVE join
            for t in range(PRE):
                if t >= 2:
                    load_wd(t)
                trend = make_trend(t)
                for j in range(JB):
                    psb = psum.tile([128, D], dt.float32, tag="ps", name="psb")
                    for k in range(KC):
                        nc.tensor.matmul(
                            psb, trend[:, k, j * 128:(j + 1) * 128],
                            wd_tiles[t][:, k],
                            start=(k == 0), stop=(k == KC - 1),
                        )
                    osb, slot = get_osb(t, j)
                    nc.vector.scalar_tensor_tensor(
                        slot, psb, 1.0, outa_pre.pop((t, j)),
                        mybir.AluOpType.mult, mybir.AluOpType.add,
                    )
                    store_out(t, j, osb)

            # ---- steady state: one 8-MM group per (t, j), ACT drains to f16
            for t in range(PRE, T):
                load_ws(t)
                load_wd(t)
                trend = make_trend(t)
                for j in range(JB):
                    ps = psum.tile([128, D], dt.float32, tag="ps", name="ps")
                    for k in range(KC):
                        nc.tensor.matmul(
                            ps, xsb[:, t, k, j * 128:(j + 1) * 128],
                            ws_tiles[t][:, k],
                            start=(k == 0), stop=False,
                        )
                    for k in range(KC):
                        nc.tensor.matmul(
                            ps, trend[:, k, j * 128:(j + 1) * 128],
                            wd_tiles[t][:, k],
                            start=False, stop=(k == KC - 1),
                        )
                    osb, slot = get_osb(t, j)
                    nc.scalar.copy(slot, ps)
                    store_out(t, j, osb)
    nc.compile()
    return nc


_NC_CACHE = {}


def _get_nc(mode="bf16"):
    if "nc" not in _NC_CACHE:
        _NC_CACHE["nc"] = build()
    return _NC_CACHE["nc"]


MODE = "bf16"


def kernel(x, W_seasonal, b_seasonal, W_trend, b_trend, _trace=False):
    npdt = ml_dtypes.bfloat16
    nc = _get_nc()

    def to_tpkd(w):  # [T, D, C] -> [T, 128, KC, D] (c-major on partitions)
        wt = w.transpose(0, 2, 1).reshape(T, KC, 128, D)
        return np.ascontiguousarray(wt.transpose(0, 2, 1, 3))

    wst = to_tpkd(W_seasonal).astype(npdt)
    wdt = to_tpkd((W_trend - W_seasonal) / 37.0).astype(npdt)
    bias = (b_seasonal + b_trend).astype(np.float32)  # host epilogue

    in_maps = []
    for i in range(NCORES):
        xs = x[i * BC:(i + 1) * BC]                    # [BC, T, C]
        # [T, C, BC] -> [T, 128, KC, BC]
        xti = xs.transpose(1, 2, 0).reshape(T, KC, 128, BC)
        xti = np.ascontiguousarray(xti.transpose(0, 2, 1, 3)).astype(npdt)
        in_maps.append({"xt": xti, "wst": wst, "wdt": wdt})

    res = run_bass_kernel_spmd(
        nc, in_maps, core_ids=list(range(NCORES)), trace=_trace
    )
    outp = np.concatenate([r["out"] for r in res.results], axis=0)
    outp = outp.astype(np.float32)
    outp += bias[None]
    if _trace:
        return outp, res
    return outp


if __name__ == "__main__":
    rng = np.random.default_rng(0)
    x = rng.standard_normal((B, T, C), dtype=np.float32)
    Ws = rng.uniform(-0.04, 0.04, (T, D, C)).astype(np.float32)
    Wt = rng.uniform(-0.04, 0.04, (T, D, C)).astype(np.float32)
    bs = rng.uniform(-0.04, 0.04, (T, D)).astype(np.float32)
    bt = rng.uniform(-0.04, 0.04, (T, D)).astype(np.float32)
    o = kernel(x, Ws, bs, Wt, bt)
    print("out shape:", o.shape, o.dtype)
